# revision 1
# baseline (speedup 1.0000x reference)
"""MLA (low-rank QKV projection + GQA attention) Bass kernel for 8 trn2 cores.

Problem shapes (hardcoded):
  x [B=2, T=2048, D=2048], Wq1 [512,2048], Wq2 [2048,512],
  Wk1/Wv1 [256,2048], Wk2/Wv2 [512,256], Wo [2048,2048]
  HQ=16 q-heads, HKV=4 kv-heads (GROUP=4), DH=128.

Sharding: core c in [0,8) owns q-heads {2c, 2c+1} and kv-head c//2.
Host folds (Wq2_head @ Wq1) etc. into per-head direct projections (rank fold
is exact math, done in float64), pre-transposes x to [D, B*T], and sums the
8 per-core partial Wo outputs.

Device per core:
  phase 1: qT [256,4096], kT [128,4096], vT [128,4096] = W.T-folded @ xT
  phase 1.5: PE-transpose vT -> v [kk-part, dd]
  phase 2: per (head, batch, 512-query chunk): scores^T = kT.T @ qT chunk,
           E = exp(scale*S) (ACT), PV accumulate + ones-row sumexp matmul,
           store unnormalized attnT, reciprocal of sumexp
  phase 3: PE-transpose recip rows into per-partition layout
  phase 4: out_partial[bt, dm] = sum_h (attnT_h.T @ WoT_h) * recip_h[bt]
"""

import os
import numpy as np

import concourse.bass as bass
import concourse.tile as tile
from concourse import mybir
from concourse import bass_utils
from concourse.vector_clock import ScopedClock

D_MODEL, HQ, HKV, RQ, RKV = 2048, 16, 4, 512, 256
DH = D_MODEL // HQ            # 128
GROUP = HQ // HKV             # 4
B, T = 2, 2048
BT = B * T                    # 4096
NCORES = 8
HPC = HQ // NCORES            # 2 q-heads per core
SCALE = 1.0 / np.sqrt(DH)

NK = D_MODEL // 128           # 16 contraction tiles over D
NBT = BT // 512               # 8 bt chunks of 512
NTT = BT // 128               # 32 bt tiles of 128
NQC = T // 512                # 4 query chunks per batch
NKK = T // 128                # 16 key tiles per batch

f32 = mybir.dt.float32


class _TC(tile.TileContext):
    pass


class _StopBuild(Exception):
    pass


_nop_ctr = [0]


def _split_multi_waits(nc):
    """This walrus build's CoreV3 lowering accepts only ONE sync-wait per
    instruction; move extra waits onto same-engine single-wait nops inserted
    immediately before the instruction."""
    for f in nc.m.functions:
        for bb in f.blocks:
            insts = list(bb.instructions)
            out = []
            changed = False
            for ins in insts:
                si = ins.sync_info
                if si is not None and si.on_wait and len(si.on_wait) > 1:
                    waits = list(si.on_wait)
                    for w in waits[:-1]:
                        _nop_ctr[0] += 1
                        nop = mybir.InstNoOp(
                            name=f"waitsplit_{_nop_ctr[0]}",
                            ins=[],
                            outs=[],
                            engine=ins.engine,
                        )
                        nop.sync_info = mybir.SyncInfo(on_wait=[w], on_update=[])
                        nc.register_instruction(nop)
                        out.append(nop)
                    ins.sync_info = mybir.SyncInfo(
                        on_wait=[waits[-1]], on_update=list(si.on_update)
                    )
                    changed = True
                out.append(ins)
            if changed:
                bb.instructions = out


def _build(mmdt, use_mask, phases=4):
    nc = bass.Bass(trn_type="TRN2")
    xT = nc.dram_tensor("xT", (D_MODEL, BT), mmdt, kind="ExternalInput")
    wq = nc.dram_tensor("wq", (D_MODEL, HPC * DH), mmdt, kind="ExternalInput")
    wk = nc.dram_tensor("wk", (D_MODEL, DH), mmdt, kind="ExternalInput")
    wv = nc.dram_tensor("wv", (D_MODEL, DH), mmdt, kind="ExternalInput")
    woT = nc.dram_tensor("woT", (HPC * DH, D_MODEL), mmdt, kind="ExternalInput")
    ones = nc.dram_tensor("ones", (128, 1), mmdt, kind="ExternalInput")
    identm = nc.dram_tensor("identm", (128, 128), mmdt, kind="ExternalInput")
    identf = nc.dram_tensor("identf", (128, 128), f32, kind="ExternalInput")
    if use_mask:
        # pre-transposed, pre-scaled by sqrt(DH): [k, q]
        maskT = nc.dram_tensor("maskT", (T, T), f32, kind="ExternalInput")
    out = nc.dram_tensor("out", (BT, D_MODEL), f32, kind="ExternalOutput")

    nc._xT, nc._wq, nc._wk, nc._wv, nc._woT = xT, wq, wk, wv, woT
    nc._ones, nc._identm, nc._identf, nc._out = ones, identm, identf, out
    if use_mask:
        nc._maskT = maskT
    try:
        _run_build(nc, mmdt, use_mask, phases)
    except _StopBuild:
        pass
    _split_multi_waits(nc)
    return nc


def _run_build(nc, mmdt, use_mask, phases):
    xT = nc._xT; wq = nc._wq; wk = nc._wk; wv = nc._wv; woT = nc._woT
    ones = nc._ones; identm = nc._identm; identf = nc._identf
    maskT = getattr(nc, "_maskT", None); out = nc._out
    Exp = mybir.ActivationFunctionType.Exp
    Copy = mybir.ActivationFunctionType.Copy
    with _TC(nc) as tc:
        with (
            tc.tile_pool(name="persist", bufs=1) as persist,
            tc.tile_pool(name="consts", bufs=1) as consts,
        ):
            qT_s = persist.tile([128, HPC * BT], mmdt)      # head h at cols h*BT
            kT_s = persist.tile([128, BT], mmdt)
            v_s = persist.tile([128, BT], mmdt)             # kk-tile t at cols t*128
            attnT_s = persist.tile([128, HPC * BT], mmdt)   # unnormalized
            recip_s = persist.tile([64, BT], f32)  # head h row at partition h*32
            rT_s = persist.tile([128, NTT * HPC], f32)
            ones_s = consts.tile([128, 1], mmdt)
            identm_s = consts.tile([128, 128], mmdt)
            identf_s = consts.tile([128, 128], f32)
            nc.sync.dma_start(ones_s[:], ones[:])
            nc.sync.dma_start(identm_s[:], identm[:])
            nc.sync.dma_start(identf_s[:], identf[:])

            # ---------------- phase 1: QKV projections ----------------
            with tc.tile_pool(name="vt", bufs=1) as vtp:
                vT_s = vtp.tile([128, BT], mmdt)
                with (
                    tc.tile_pool(name="wgt", bufs=1) as wgt,
                    tc.tile_pool(name="xin", bufs=3) as xin,
                    tc.tile_pool(name="qkvp", bufs=2, space="PSUM") as qkvp,
                ):
                    wq_s = wgt.tile([128, NK * HPC * DH], mmdt)
                    wk_s = wgt.tile([128, NK * DH], mmdt)
                    wv_s = wgt.tile([128, NK * DH], mmdt)
                    nc.sync.dma_start(
                        wq_s[:].rearrange("p (t m) -> p t m", t=NK),
                        wq[:].rearrange("(t p) m -> p t m", p=128),
                    )
                    nc.sync.dma_start(
                        wk_s[:].rearrange("p (t m) -> p t m", t=NK),
                        wk[:].rearrange("(t p) m -> p t m", p=128),
                    )
                    nc.sync.dma_start(
                        wv_s[:].rearrange("p (t m) -> p t m", t=NK),
                        wv[:].rearrange("(t p) m -> p t m", p=128),
                    )
                    for n in range(NBT):
                        ps_q0 = qkvp.tile([128, 512], f32, tag="psq0")
                        ps_q1 = qkvp.tile([128, 512], f32, tag="psq1")
                        ps_k = qkvp.tile([128, 512], f32, tag="psk")
                        ps_v = qkvp.tile([128, 512], f32, tag="psv")
                        for kd in range(NK):
                            xt = xin.tile([128, 512], mmdt, tag="xt")
                            nc.sync.dma_start(
                                xt[:],
                                xT[kd * 128 : (kd + 1) * 128, n * 512 : (n + 1) * 512],
                            )
                            st, sp = kd == 0, kd == NK - 1
                            nc.tensor.matmul(
                                ps_q0[:], wq_s[:, kd * 256 : kd * 256 + 128], xt[:],
                                start=st, stop=sp,
                            )
                            nc.tensor.matmul(
                                ps_q1[:], wq_s[:, kd * 256 + 128 : kd * 256 + 256], xt[:],
                                start=st, stop=sp,
                            )
                            nc.tensor.matmul(
                                ps_k[:], wk_s[:, kd * 128 : (kd + 1) * 128], xt[:],
                                start=st, stop=sp,
                            )
                            nc.tensor.matmul(
                                ps_v[:], wv_s[:, kd * 128 : (kd + 1) * 128], xt[:],
                                start=st, stop=sp,
                            )
                        sl = slice(n * 512, (n + 1) * 512)
                        nc.vector.tensor_copy(qT_s[:, n * 512 : (n + 1) * 512], ps_q0[:])
                        nc.vector.tensor_copy(
                            qT_s[:, BT + n * 512 : BT + (n + 1) * 512], ps_q1[:]
                        )
                        nc.scalar.activation(kT_s[:, sl], ps_k[:], Copy)
                        nc.scalar.activation(vT_s[:, sl], ps_v[:], Copy)

                # -------- phase 1.5: transpose vT -> v (kk on partitions) ----
                with tc.tile_pool(name="trp", bufs=4, space="PSUM") as trp:
                    for t in range(NTT):
                        tr = trp.tile([128, 128], mmdt, tag="tr")
                        nc.tensor.transpose(
                            tr[:], vT_s[:, t * 128 : (t + 1) * 128], identm_s[:]
                        )
                        nc.vector.tensor_copy(v_s[:, t * 128 : (t + 1) * 128], tr[:])

            # ---------------- phase 2: attention ----------------
            if phases >= 2:
                with (
                    tc.tile_pool(name="epool", bufs=20) as epool,
                    tc.tile_pool(name="mpool", bufs=3) as mpool,
                    tc.tile_pool(name="stp", bufs=4, space="PSUM") as stp,
                    tc.tile_pool(name="pvp", bufs=2, space="PSUM") as pvp,
                    tc.tile_pool(name="sump", bufs=2, space="PSUM") as sump,
                ):
                    chunks = [
                        (h, b, qc)
                        for h in range(HPC)
                        for b in range(B)
                        for qc in range(NQC)
                    ]

                    def emit_scores(ci, kk):
                        h, b, qc = chunks[ci]
                        qsl = qT_s[
                            :,
                            h * BT + b * T + qc * 512 : h * BT + b * T + (qc + 1) * 512,
                        ]
                        ps_st = stp.tile([128, 512], f32, tag="st", name=f"st_{ci}_{kk}")
                        nc.tensor.matmul(
                            ps_st[:],
                            kT_s[:, b * T + kk * 128 : b * T + (kk + 1) * 128],
                            qsl,
                            start=True, stop=True,
                        )
                        if use_mask:
                            mt = mpool.tile([128, 512], f32, tag="mt", name=f"mt_{ci}_{kk}")
                            nc.sync.dma_start(
                                mt[:],
                                maskT[
                                    kk * 128 : (kk + 1) * 128,
                                    qc * 512 : (qc + 1) * 512,
                                ],
                            )
                            nc.vector.tensor_add(ps_st[:], ps_st[:], mt[:])
                        e = epool.tile([128, 512], mmdt, tag="e", name=f"e_{ci}_{kk}")
                        nc.scalar.activation(e[:], ps_st[:], Exp, scale=SCALE)
                        return e

                    # software pipeline: chunk ci's PV/ones MMs interleave with
                    # chunk ci+1's score MMs so PE never waits on ACT exp.
                    es_cur = [emit_scores(0, kk) for kk in range(NKK)]
                    for ci in range(len(chunks)):
                        h, b, qc = chunks[ci]
                        ps_pv = pvp.tile([128, 512], f32, tag="pv", name=f"pv_{ci}")
                        ps_sum = sump.tile([1, 512], f32, tag="sum", name=f"sum_{ci}")
                        es_next = []
                        for kk in range(NKK):
                            st, sp = kk == 0, kk == NKK - 1
                            nc.tensor.matmul(
                                ps_pv[:],
                                v_s[:, (b * NKK + kk) * 128 : (b * NKK + kk + 1) * 128],
                                es_cur[kk][:],
                                start=st, stop=sp,
                            )
                            nc.tensor.matmul(
                                ps_sum[:], ones_s[:], es_cur[kk][:],
                                start=st, stop=sp,
                            )
                            if ci + 1 < len(chunks):
                                es_next.append(emit_scores(ci + 1, kk))
                        osl = slice(b * T + qc * 512, b * T + (qc + 1) * 512)
                        nc.vector.reciprocal(
                            recip_s[h * 32 : h * 32 + 1, osl], ps_sum[0:1, :]
                        )
                        nc.scalar.activation(
                            attnT_s[:, h * BT + b * T + qc * 512 : h * BT + b * T + (qc + 1) * 512],
                            ps_pv[:],
                            Copy,
                        )
                        es_cur = es_next

                # ---------------- phase 3: transpose recip rows ----------------
            if phases >= 3:
                with tc.tile_pool(name="rtp", bufs=4, space="PSUM") as rtp:
                    for i in range(NTT):
                        tr = rtp.tile([128, 64], f32, tag="rtr")
                        nc.tensor.transpose(
                            tr[:],
                            recip_s[0:64, i * 128 : (i + 1) * 128],
                            identf_s[0:64, 0:64],
                        )
                        nc.vector.tensor_copy(rT_s[:, i * HPC : i * HPC + 1], tr[:, 0:1])
                        nc.vector.tensor_copy(
                            rT_s[:, i * HPC + 1 : i * HPC + 2], tr[:, 32:33]
                        )

                # ---------------- phase 4: Wo partial ----------------
            if phases >= 4:
                with (
                    tc.tile_pool(name="wop", bufs=1) as wop,
                    tc.tile_pool(name="omg", bufs=4) as omg,
                    tc.tile_pool(name="wops", bufs=4, space="PSUM") as wops,
                ):
                    woT_s = wop.tile([128, HPC * D_MODEL], mmdt)
                    for h in range(HPC):
                        nc.sync.dma_start(
                            woT_s[:, h * D_MODEL : (h + 1) * D_MODEL],
                            woT[h * 128 : (h + 1) * 128, :],
                        )
                    for i in range(NTT):
                        for dc in range(4):
                            p0 = wops.tile([128, 512], f32, tag="p0")
                            p1 = wops.tile([128, 512], f32, tag="p1")
                            nc.tensor.matmul(
                                p0[:],
                                attnT_s[:, 0 * BT + i * 128 : 0 * BT + (i + 1) * 128],
                                woT_s[:, 0 * D_MODEL + dc * 512 : 0 * D_MODEL + (dc + 1) * 512],
                                start=True, stop=True,
                            )
                            nc.tensor.matmul(
                                p1[:],
                                attnT_s[:, 1 * BT + i * 128 : 1 * BT + (i + 1) * 128],
                                woT_s[:, 1 * D_MODEL + dc * 512 : 1 * D_MODEL + (dc + 1) * 512],
                                start=True, stop=True,
                            )
                            t0 = omg.tile([128, 512], f32, tag="t0")
                            t1 = omg.tile([128, 512], f32, tag="t1")
                            nc.scalar.activation(
                                t0[:], p0[:], Copy, scale=rT_s[:, i * HPC : i * HPC + 1]
                            )
                            nc.vector.tensor_scalar_mul(
                                t1[:], p1[:], rT_s[:, i * HPC + 1 : i * HPC + 2]
                            )
                            oo = omg.tile([128, 512], f32, tag="oo")
                            nc.vector.tensor_add(oo[:], t0[:], t1[:])
                            nc.sync.dma_start(
                                out[i * 128 : (i + 1) * 128, dc * 512 : (dc + 1) * 512],
                                oo[:],
                            )
        _split_multi_waits(nc)
        return nc


_cache = {}


def _get_nc(mmdt_name, use_mask):
    phases = int(os.environ.get("BASS_MLA_PHASES", "4"))
    key = (mmdt_name, use_mask, phases)
    if key not in _cache:
        _cache[key] = _build(getattr(mybir.dt, mmdt_name), use_mask, phases)
    return _cache[key]


def _np_dt(mmdt_name):
    if mmdt_name == "bfloat16":
        import ml_dtypes

        return ml_dtypes.bfloat16
    return np.float32


def _prep_inputs(x, attn_mask, Wq1, Wq2, Wk1, Wk2, Wv1, Wv2, Wo, mmdt_name):
    ndt = _np_dt(mmdt_name)
    xT = np.ascontiguousarray(x.reshape(BT, D_MODEL).T).astype(ndt)
    identm = np.eye(128, dtype=np.float32).astype(ndt)
    identf = np.eye(128, dtype=np.float32)
    ones = np.ones((128, 1), np.float32).astype(ndt)
    use_mask = bool(np.any(attn_mask))
    maskT = None
    if use_mask:
        maskT = np.ascontiguousarray(attn_mask[0, 0].T * np.sqrt(DH)).astype(
            np.float32
        )
    Wq1_64, Wq2_64 = Wq1.astype(np.float64), Wq2.astype(np.float64)
    Wk1_64, Wk2_64 = Wk1.astype(np.float64), Wk2.astype(np.float64)
    Wv1_64, Wv2_64 = Wv1.astype(np.float64), Wv2.astype(np.float64)
    in_maps = []
    for c in range(NCORES):
        h0 = c * HPC
        kv = h0 // GROUP
        wq_f = (Wq2_64[h0 * DH : (h0 + HPC) * DH] @ Wq1_64).T  # [D, HPC*DH]
        wk_f = (Wk2_64[kv * DH : (kv + 1) * DH] @ Wk1_64).T    # [D, DH]
        wv_f = (Wv2_64[kv * DH : (kv + 1) * DH] @ Wv1_64).T
        woT_c = np.ascontiguousarray(Wo[:, h0 * DH : (h0 + HPC) * DH].T)
        m = {
            "xT": xT,
            "wq": np.ascontiguousarray(wq_f).astype(ndt),
            "wk": np.ascontiguousarray(wk_f).astype(ndt),
            "wv": np.ascontiguousarray(wv_f).astype(ndt),
            "woT": woT_c.astype(ndt),
            "ones": ones,
            "identm": identm,
            "identf": identf,
        }
        if use_mask:
            m["maskT"] = maskT
        in_maps.append(m)
    return in_maps, use_mask


def run(x, attn_mask, Wq1, Wq2, Wk1, Wk2, Wv1, Wv2, Wo, **spmd_kwargs):
    mmdt_name = os.environ.get("BASS_MLA_DT", "float32r")
    in_maps, use_mask = _prep_inputs(
        x, attn_mask, Wq1, Wq2, Wk1, Wk2, Wv1, Wv2, Wo, mmdt_name
    )
    nc = _get_nc(mmdt_name, use_mask)
    res = bass_utils.run_bass_kernel_spmd(
        nc, in_maps, core_ids=list(range(NCORES)), **spmd_kwargs
    )
    acc = res.results[0]["out"].astype(np.float64)
    for r in res.results[1:]:
        acc += r["out"]
    out = acc.astype(np.float32).reshape(B, T, D_MODEL)
    return out, res


def kernel(x, attn_mask, Wq1, Wq2, Wk1, Wk2, Wv1, Wv2, Wo):
    out, _ = run(x, attn_mask, Wq1, Wq2, Wk1, Wk2, Wv1, Wv2, Wo)
    return out



# revision 10
# speedup vs baseline: 1.3848x; 1.3848x over previous
"""MLA (low-rank QKV projection + GQA attention) Bass kernel for 8 trn2 cores.

Problem shapes (hardcoded):
  x [B=2, T=2048, D=2048], Wq1 [512,2048], Wq2 [2048,512],
  Wk1/Wv1 [256,2048], Wk2/Wv2 [512,256], Wo [2048,2048]
  HQ=16 q-heads, HKV=4 kv-heads (GROUP=4), DH=128.

Sharding: core c = (b, g) with b = c//4 (data-parallel over batch),
g = c%4 (tensor-parallel over head groups). Each core owns q-heads
{4g..4g+3} and kv-head g for its batch's 2048 tokens. Host folds
(W2_head @ W1) into per-head direct projections (exact math in float64),
pre-transposes x[b] to [D, T], and sums the 4 per-core partial Wo
outputs per batch.

Device per core:
  phase 1: qT [128, 4*2048], kT [128, 2048], vT [128, 2048] via folded
           weights; vT PE-transposed into v (key tokens on partitions),
           interleaved with the projection loop.
  phase 2: per (qc, h) chunk of 512 queries: scores^T = kT.T @ qT chunk,
           E = exp(scale*S) (ACT), PV accumulate + ones-row sumexp
           matmul, store unnormalized attnT (bf16), sumexp rows.
           Chunks are software-pipelined (next chunk's scores interleave
           with this chunk's PV) and ordered qc-major so phases 3/4 for
           qc can start while qc+1 is still in attention.
  phase 3: per qc: PE-transpose sumexp rows [4,128] -> [128,4], vector
           reciprocal -> rT (per-token-per-head normalizers).
  phase 4: per (token tile, dmodel chunk): 4 per-head Wo matmuls, then a
           fused scale-and-accumulate chain spread over scalar/vector/
           gpsimd engines, normalized output DMA'd to DRAM.
"""

import os
import numpy as np

import concourse.bass as bass
import concourse.tile as tile
from concourse import mybir
from concourse import bass_utils

D_MODEL, HQ, HKV, RQ, RKV = 2048, 16, 4, 512, 256
DH = D_MODEL // HQ            # 128
GROUP = HQ // HKV             # 4
B, T = 2, 2048
NCORES = 8
NGROUP = 4                    # tensor-parallel groups (one per kv head)
HPC = HQ // NGROUP            # 4 q-heads per core
SCALE = 1.0 / np.sqrt(DH)

NK = D_MODEL // 128           # 16 contraction tiles over D
NTC = T // 512                # 4 token chunks of 512
NKK = T // 128                # 16 key tiles of 128
NQC = T // 512                # 4 query chunks of 512
NTT = T // 128                # 16 token tiles of 128

f32 = mybir.dt.float32


class _TC(tile.TileContext):
    pass


_nop_ctr = [0]


def _split_multi_waits(nc):
    """This walrus build's CoreV3 lowering accepts only ONE sync-wait per
    instruction; move extra waits onto same-engine single-wait nops inserted
    immediately before the instruction."""
    for f in nc.m.functions:
        for bb in f.blocks:
            insts = list(bb.instructions)
            out = []
            changed = False
            for ins in insts:
                si = ins.sync_info
                if si is not None and si.on_wait and len(si.on_wait) > 1:
                    waits = list(si.on_wait)
                    for w in waits[:-1]:
                        _nop_ctr[0] += 1
                        nop = mybir.InstNoOp(
                            name=f"waitsplit_{_nop_ctr[0]}",
                            ins=[],
                            outs=[],
                            engine=ins.engine,
                        )
                        nop.sync_info = mybir.SyncInfo(on_wait=[w], on_update=[])
                        nc.register_instruction(nop)
                        out.append(nop)
                    ins.sync_info = mybir.SyncInfo(
                        on_wait=[waits[-1]], on_update=list(si.on_update)
                    )
                    changed = True
                out.append(ins)
            if changed:
                bb.instructions = out


def _build(mmdt, use_mask):
    nc = bass.Bass(trn_type="TRN2")
    xT = nc.dram_tensor("xT", (D_MODEL, T), mmdt, kind="ExternalInput")
    wq = nc.dram_tensor("wq", (D_MODEL, HPC * DH), mmdt, kind="ExternalInput")
    wk = nc.dram_tensor("wk", (D_MODEL, DH), mmdt, kind="ExternalInput")
    wv = nc.dram_tensor("wv", (D_MODEL, DH), mmdt, kind="ExternalInput")
    woT = nc.dram_tensor("woT", (HPC * DH, D_MODEL), mmdt, kind="ExternalInput")
    ones = nc.dram_tensor("ones", (128, 1), mmdt, kind="ExternalInput")
    identm = nc.dram_tensor("identm", (128, 128), mmdt, kind="ExternalInput")
    identf = nc.dram_tensor("identf", (128, 128), f32, kind="ExternalInput")
    if use_mask:
        # pre-transposed, pre-scaled by sqrt(DH): [k, q]
        maskT = nc.dram_tensor("maskT", (T, T), f32, kind="ExternalInput")
    else:
        maskT = None
    out = nc.dram_tensor("out", (T, D_MODEL), f32, kind="ExternalOutput")

    Exp = mybir.ActivationFunctionType.Exp
    Copy = mybir.ActivationFunctionType.Copy
    Mult = mybir.AluOpType.mult
    Add = mybir.AluOpType.add

    with _TC(nc) as tc:
        with (
            tc.tile_pool(name="persist", bufs=1) as persist,
            tc.tile_pool(name="consts", bufs=1) as consts,
        ):
            qT_s = persist.tile([128, HPC * T], mmdt)     # head h at cols h*T
            kT_s = persist.tile([128, T], mmdt)
            vT_s = persist.tile([128, T], mmdt)
            v_s = persist.tile([128, T], mmdt)            # kk-tile t at cols t*128
            attnT_s = persist.tile([128, HPC * T], mmdt)  # unnormalized PV
            sumexp_s = persist.tile([128, T], f32)        # head h on partition 32*h
            rT_s = persist.tile([128, NTT * HPC], f32)    # recip, tok on partition
            woT_s = persist.tile([128, HPC * D_MODEL], mmdt)
            ones_s = consts.tile([128, 1], mmdt)
            identm_s = consts.tile([128, 128], mmdt)
            identf_s = consts.tile([128, 128], f32)
            # unused partitions of sumexp_s flow through the phase-3
            # transpose; init so no garbage/non-finite values are read
            nc.vector.memset(sumexp_s[:], 1.0)

            # ---------------- phase 1: QKV projections ----------------
            # Weight/const DMA goes on the scalar queue (sync queue carries
            # x tiles); per-kd slices so the first matmuls start early.
            with (
                tc.tile_pool(name="wgt", bufs=1) as wgt,
                tc.tile_pool(name="xin", bufs=4) as xin,
                tc.tile_pool(name="qkvp", bufs=1, space="PSUM") as qkvp,
                tc.tile_pool(name="trp", bufs=2, space="PSUM") as trp,
            ):
                wq_s = wgt.tile([128, NK * HPC * DH], mmdt)
                wk_s = wgt.tile([128, NK * DH], mmdt)
                wv_s = wgt.tile([128, NK * DH], mmdt)
                for kd in range(NK):
                    ksl = slice(kd * 128, (kd + 1) * 128)
                    nc.scalar.dma_start(
                        wq_s[:, kd * 512 : (kd + 1) * 512], wq[ksl, :]
                    )
                    nc.scalar.dma_start(
                        wk_s[:, kd * 128 : (kd + 1) * 128], wk[ksl, :]
                    )
                    nc.scalar.dma_start(
                        wv_s[:, kd * 128 : (kd + 1) * 128], wv[ksl, :]
                    )
                    if kd == 2:
                        nc.scalar.dma_start(ones_s[:], ones[:])
                        nc.scalar.dma_start(identm_s[:], identm[:])
                        nc.scalar.dma_start(identf_s[:], identf[:])
                for h in range(HPC):
                    nc.scalar.dma_start(
                        woT_s[:, h * D_MODEL : (h + 1) * D_MODEL],
                        woT[h * 128 : (h + 1) * 128, :],
                    )

                def emit_vtr(n):
                    # transpose vT chunk n (4 key tiles) into v_s
                    for t in range(n * 4, n * 4 + 4):
                        tr = trp.tile([128, 128], mmdt, tag="tr", name=f"tr_{t}")
                        nc.tensor.transpose(
                            tr[:], vT_s[:, t * 128 : (t + 1) * 128], identm_s[:]
                        )
                        nc.vector.tensor_copy(v_s[:, t * 128 : (t + 1) * 128], tr[:])

                for n in range(NTC):
                    nsl = slice(n * 512, (n + 1) * 512)
                    ps_q = [
                        qkvp.tile([128, 512], f32, tag=f"psq{j}", name=f"psq{j}_{n}")
                        for j in range(HPC)
                    ]
                    ps_k = qkvp.tile([128, 512], f32, tag="psk", name=f"psk_{n}")
                    ps_v = qkvp.tile([128, 512], f32, tag="psv", name=f"psv_{n}")
                    for kd in range(NK):
                        xt = xin.tile([128, 512], mmdt, tag="xt", name=f"xt_{n}_{kd}")
                        nc.sync.dma_start(
                            xt[:], xT[kd * 128 : (kd + 1) * 128, nsl]
                        )
                        st, sp = kd == 0, kd == NK - 1
                        for j in range(HPC):
                            nc.tensor.matmul(
                                ps_q[j][:],
                                wq_s[:, kd * 512 + j * 128 : kd * 512 + (j + 1) * 128],
                                xt[:],
                                start=st, stop=sp,
                            )
                        nc.tensor.matmul(
                            ps_k[:], wk_s[:, kd * 128 : (kd + 1) * 128], xt[:],
                            start=st, stop=sp,
                        )
                        nc.tensor.matmul(
                            ps_v[:], wv_s[:, kd * 128 : (kd + 1) * 128], xt[:],
                            start=st, stop=sp,
                        )
                    # drain psums (gpsimd cannot read PSUM; scalar/vector only)
                    nc.scalar.activation(qT_s[:, 0 * T + n * 512 : 0 * T + (n + 1) * 512], ps_q[0][:], Copy)
                    nc.scalar.activation(qT_s[:, 1 * T + n * 512 : 1 * T + (n + 1) * 512], ps_q[1][:], Copy)
                    nc.vector.tensor_copy(qT_s[:, 2 * T + n * 512 : 2 * T + (n + 1) * 512], ps_q[2][:])
                    nc.vector.tensor_copy(qT_s[:, 3 * T + n * 512 : 3 * T + (n + 1) * 512], ps_q[3][:])
                    nc.vector.tensor_copy(kT_s[:, nsl], ps_k[:])
                    nc.vector.tensor_copy(vT_s[:, nsl], ps_v[:])
                    if n > 0:
                        emit_vtr(n - 1)
                emit_vtr(NTC - 1)

            # ---------------- phases 2+3+4 interleaved ----------------
            with (
                tc.tile_pool(name="epool", bufs=36) as epool,
                tc.tile_pool(name="omg", bufs=6) as omg,
                tc.tile_pool(name="mpool", bufs=3) as mpool,
                tc.tile_pool(name="stp", bufs=2, space="PSUM") as stp,
                tc.tile_pool(name="pvp", bufs=1, space="PSUM") as pvp,
                tc.tile_pool(name="sump", bufs=1, space="PSUM") as sump,
                tc.tile_pool(name="wops", bufs=1, space="PSUM") as wops,
            ):
                chunks = [(qc, h) for qc in range(NQC) for h in range(HPC)]

                def emit_scores(ci, kt):
                    qc, h = chunks[ci]
                    qsl = qT_s[:, h * T + qc * 512 : h * T + (qc + 1) * 512]
                    ps_st = stp.tile([128, 512], f32, tag="st", name=f"st_{ci}_{kt}")
                    nc.tensor.matmul(
                        ps_st[:],
                        kT_s[:, kt * 128 : (kt + 1) * 128],
                        qsl,
                        start=True, stop=True,
                    )
                    if use_mask:
                        mt = mpool.tile([128, 512], f32, tag="mt", name=f"mt_{ci}_{kt}")
                        nc.sync.dma_start(
                            mt[:],
                            maskT[kt * 128 : (kt + 1) * 128, qc * 512 : (qc + 1) * 512],
                        )
                        nc.vector.tensor_add(ps_st[:], ps_st[:], mt[:])
                    e = epool.tile([128, 512], mmdt, tag="e", name=f"e_{ci}_{kt}")
                    nc.scalar.activation(e[:], ps_st[:], Exp, scale=SCALE)
                    return e

                def emit_p4_item(it, dc):
                    # 4 per-head Wo matmuls + fused normalize-and-combine
                    ps = []
                    for h in range(HPC):
                        p = wops.tile([128, 512], f32, tag=f"wo{h}", name=f"wo{h}_{it}_{dc}")
                        nc.tensor.matmul(
                            p[:],
                            attnT_s[:, h * T + it * 128 : h * T + (it + 1) * 128],
                            woT_s[:, h * D_MODEL + dc * 512 : h * D_MODEL + (dc + 1) * 512],
                            start=True, stop=True,
                        )
                        ps.append(p)
                    r = lambda h: rT_s[:, it * HPC + h : it * HPC + h + 1]
                    t0 = omg.tile([128, 512], f32, tag="t0", name=f"t0_{it}_{dc}")
                    nc.scalar.activation(t0[:], ps[0][:], Copy, scale=r(0))
                    t1 = omg.tile([128, 512], f32, tag="t1", name=f"t1_{it}_{dc}")
                    nc.vector.scalar_tensor_tensor(t1[:], ps[1][:], r(1), t0[:], Mult, Add)
                    t2 = omg.tile([128, 512], f32, tag="t2", name=f"t2_{it}_{dc}")
                    nc.vector.scalar_tensor_tensor(t2[:], ps[2][:], r(2), t1[:], Mult, Add)
                    oo = omg.tile([128, 512], f32, tag="oo", name=f"oo_{it}_{dc}")
                    nc.vector.scalar_tensor_tensor(oo[:], ps[3][:], r(3), t2[:], Mult, Add)
                    nc.sync.dma_start(
                        out[it * 128 : (it + 1) * 128, dc * 512 : (dc + 1) * 512],
                        oo[:],
                    )

                p4q = []  # deferred (it, dc) work items
                es_cur = [emit_scores(0, kt) for kt in range(NKK)]
                for ci in range(len(chunks)):
                    qc, h = chunks[ci]
                    ps_pv = pvp.tile([128, 512], f32, tag="pv", name=f"pv_{ci}")
                    ps_sum = sump.tile([1, 512], f32, tag="sum", name=f"sum_{ci}")
                    es_next = []
                    for kt in range(NKK):
                        st, sp = kt == 0, kt == NKK - 1
                        nc.tensor.matmul(
                            ps_pv[:],
                            v_s[:, kt * 128 : (kt + 1) * 128],
                            es_cur[kt][:],
                            start=st, stop=sp,
                        )
                        nc.tensor.matmul(
                            ps_sum[:], ones_s[:], es_cur[kt][:],
                            start=st, stop=sp,
                        )
                        if sp:
                            # free pv/sum banks as soon as accumulation stops
                            nc.vector.tensor_copy(
                                attnT_s[:, h * T + qc * 512 : h * T + (qc + 1) * 512],
                                ps_pv[:],
                            )
                            nc.scalar.activation(
                                sumexp_s[32 * h : 32 * h + 1, qc * 512 : (qc + 1) * 512],
                                ps_sum[0:1, :],
                                Copy,
                            )
                        if ci + 1 < len(chunks):
                            es_next.append(emit_scores(ci + 1, kt))
                        if p4q and kt % 4 == 2:
                            emit_p4_item(*p4q.pop(0))
                    es_cur = es_next
                    if h == HPC - 1:
                        # phase 3 for this qc: transpose sumexp rows, recip.
                        # Heads live at partitions {0,32,64,96} (engine
                        # partition-start constraint), so transpose the full
                        # 128 partitions and recip the 4 head columns.
                        for tt in range(4):
                            it = qc * 4 + tt
                            ps_r = wops.tile([128, 128], f32, tag="wo0", name=f"psr_{it}")
                            nc.tensor.transpose(
                                ps_r[:],
                                sumexp_s[:, qc * 512 + tt * 128 : qc * 512 + (tt + 1) * 128],
                                identf_s[:],
                            )
                            for hh in range(HPC):
                                nc.vector.reciprocal(
                                    rT_s[:, it * HPC + hh : it * HPC + hh + 1],
                                    ps_r[:, 32 * hh : 32 * hh + 1],
                                )
                        p4q += [(qc * 4 + tt, dc) for tt in range(4) for dc in range(4)]
                for it, dc in p4q:
                    emit_p4_item(it, dc)

    _split_multi_waits(nc)
    return nc


_cache = {}


def _get_nc(mmdt_name, use_mask):
    key = (mmdt_name, use_mask)
    if key not in _cache:
        _cache[key] = _build(getattr(mybir.dt, mmdt_name), use_mask)
    return _cache[key]


def _np_dt(mmdt_name):
    if mmdt_name == "bfloat16":
        import ml_dtypes

        return ml_dtypes.bfloat16
    return np.float32


def _prep_inputs(x, attn_mask, Wq1, Wq2, Wk1, Wk2, Wv1, Wv2, Wo, mmdt_name):
    ndt = _np_dt(mmdt_name)
    identm = np.eye(128, dtype=np.float32).astype(ndt)
    identf = np.eye(128, dtype=np.float32)
    ones = np.ones((128, 1), np.float32).astype(ndt)
    use_mask = bool(np.any(attn_mask))
    maskT = None
    if use_mask:
        maskT = np.ascontiguousarray(attn_mask[0, 0].T * np.sqrt(DH)).astype(
            np.float32
        )
    Wq1_64, Wq2_64 = Wq1.astype(np.float64), Wq2.astype(np.float64)
    Wk1_64, Wk2_64 = Wk1.astype(np.float64), Wk2.astype(np.float64)
    Wv1_64, Wv2_64 = Wv1.astype(np.float64), Wv2.astype(np.float64)
    xT_b = [
        np.ascontiguousarray(np.asarray(x[b]).T).astype(ndt) for b in range(B)
    ]
    in_maps = []
    for c in range(NCORES):
        b, g = divmod(c, NGROUP)
        h0 = g * HPC
        wq_f = (Wq2_64[h0 * DH : (h0 + HPC) * DH] @ Wq1_64).T  # [D, HPC*DH]
        wk_f = (Wk2_64[g * DH : (g + 1) * DH] @ Wk1_64).T      # [D, DH]
        wv_f = (Wv2_64[g * DH : (g + 1) * DH] @ Wv1_64).T
        woT_c = np.ascontiguousarray(Wo[:, h0 * DH : (h0 + HPC) * DH].T)
        m = {
            "xT": xT_b[b],
            "wq": np.ascontiguousarray(wq_f).astype(ndt),
            "wk": np.ascontiguousarray(wk_f).astype(ndt),
            "wv": np.ascontiguousarray(wv_f).astype(ndt),
            "woT": woT_c.astype(ndt),
            "ones": ones,
            "identm": identm,
            "identf": identf,
        }
        if use_mask:
            m["maskT"] = maskT
        in_maps.append(m)
    return in_maps, use_mask


def run(x, attn_mask, Wq1, Wq2, Wk1, Wk2, Wv1, Wv2, Wo, **spmd_kwargs):
    mmdt_name = os.environ.get("BASS_MLA_DT", "bfloat16")
    in_maps, use_mask = _prep_inputs(
        x, attn_mask, Wq1, Wq2, Wk1, Wk2, Wv1, Wv2, Wo, mmdt_name
    )
    nc = _get_nc(mmdt_name, use_mask)
    res = bass_utils.run_bass_kernel_spmd(
        nc, in_maps, core_ids=list(range(NCORES)), **spmd_kwargs
    )
    out = np.zeros((B, T, D_MODEL), np.float64)
    for c in range(NCORES):
        out[c // NGROUP] += res.results[c]["out"]
    return out.astype(np.float32), res


def kernel(x, attn_mask, Wq1, Wq2, Wk1, Wk2, Wv1, Wv2, Wo):
    out, _ = run(x, attn_mask, Wq1, Wq2, Wk1, Wk2, Wv1, Wv2, Wo)
    return out
